# revision 49
# baseline (speedup 1.0000x reference)
"""Mistral attention (B=2, S=2048, D=4096, H=32, KVH=8, HD=128) on 8 trn2 cores.

Sharding: core c -> (batch b = c//4, head-group g = c%4).
Each core computes q/k/v projections for its 8 Q heads + 2 KV heads of one
batch, RoPE, causal attention, and a row-parallel partial o_proj
[2048, 4096]. Host sums the 4 partials per batch. No collectives.

All matmul operands are bf16 (same PE rate as fp32r on trn2, half the HBM
traffic); accumulation, softmax logits and the final output stay fp32.
Attention runs transposed: scoresT[keys, qtok] with keys on partitions, so
the key-sum is a ones-matmul and AV^T produces attn_outT per head, which
feeds o_proj directly as the stationary operand.

Schedule (causal): the projections of token block th+1 are emitted as
fine-grained PE "filler" units interleaved into the attention inner loop of
block th, so the scalar-engine exp latency never idles the PE; the o_proj
contribution of block th follows its attention directly (no DRAM spill),
with half of o_proj(2) deferred into attention(3) as its filler.
"""

import itertools
import os
import sys

for _p in ("/opt/trn_rl_repo",):
    if _p not in sys.path:
        sys.path.insert(0, _p)

import ml_dtypes
import numpy as np

import concourse.bass as bass
import concourse.tile as tile
from concourse import bacc, mybir
from concourse.bass_utils import run_bass_kernel_spmd

F32 = mybir.dt.float32
F32R = mybir.dt.float32r
BF16 = mybir.dt.bfloat16
EXP = mybir.ActivationFunctionType.Exp

B, S, D = 2, 2048, 4096
H, KVH, HD = 32, 8, 128
SCALE = HD ** -0.5
NCORES = 8

QH = H // 4              # 8 q heads per core
QCOLS = QH * HD          # 1024
KCOLS = (KVH // 4) * HD  # 256 (2 kv heads per core)
TOK = S

NEG = -1e9

_DONE = object()

_PROGRAMS = {}


def _build_program(variant: str):
    """variant: 'causal' | 'zero' | 'general'"""
    nc = bacc.Bacc("TRN2", target_bir_lowering=False, debug=False)

    # [th][cchunk 32][c 128][tok 512]
    hT = nc.dram_tensor("hT", [4, 32, 128, 512], BF16, kind="ExternalInput").ap()
    # [cb 12][c 128][cchunk*out 32*128]
    wqkv = nc.dram_tensor("wqkv", [12, 128, 32 * 128], BF16, kind="ExternalInput").ap()
    # [nb 8][hc 8][c 128][out 512]
    wo = nc.dram_tensor("wo", [8, 8, 128, 512], BF16, kind="ExternalInput").ap()
    cosT = nc.dram_tensor("cosT", [HD, TOK], F32, kind="ExternalInput").ap()
    sinTr = nc.dram_tensor("sinTr", [HD, TOK], F32, kind="ExternalInput").ap()
    ident = nc.dram_tensor("ident", [128, 128], F32R, kind="ExternalInput").ap()
    ones = nc.dram_tensor("ones", [128, 1], BF16, kind="ExternalInput").ap()
    if variant == "causal":
        maskT = nc.dram_tensor("maskT", [128, 4 * 512], F32, kind="ExternalInput").ap()
    elif variant == "general":
        maskT = nc.dram_tensor("maskT", [S, S], F32, kind="ExternalInput").ap()
    else:
        maskT = None
    out = nc.dram_tensor("out", [TOK, D], BF16, kind="ExternalOutput").ap()

    if variant != "causal":
        qT_spill = nc.dram_tensor("qT_spill", [QCOLS, TOK], BF16).ap()

    NTH = 4
    THW = TOK // NTH         # 512
    NCH = D // 128           # 32 contraction chunks
    NCB = (QCOLS + 2 * KCOLS) // 128  # 12: 0-7 q, 8-9 k, 10-11 v

    with tile.TileContext(nc) as tc:
        with tc.tile_pool(name="per", bufs=1) as per, \
             tc.tile_pool(name="wrk", bufs=2) as wrk, \
             tc.tile_pool(name="one", bufs=1) as one, \
             tc.tile_pool(name="ps", bufs=2, space="PSUM") as psp:

            ident_sb = per.tile([128, 128], F32R, tag="ident")
            ones_sb = per.tile([128, 1], BF16, tag="ones")
            kT_sb = per.tile([HD, 2 * TOK], BF16, tag="kT")
            V_sb = per.tile([128, (TOK // 128) * KCOLS], BF16, tag="V")
            wo_sb = per.tile([128, 8 * 8 * 512], BF16, tag="wo")
            mask_sb = None
            if variant == "causal":
                mask_sb = per.tile([128, 4 * 512], F32, tag="mask")

            at2s = {}
            qT_store = {}

            # k/v chunks first, then q pairs: attention of a query pair can
            # start as soon as its two q chunks land, so attention(0) can
            # interleave into proj(0)'s own DMA-paced tail.
            CB_ORDER = [8, 9, 10, 11] + list(range(8))

            def proj_units(th):
                """Generator: projections for token block th in small PE
                units so the previous block's attention can interleave them
                as filler. First proj-weight chunk leads the DMA queues."""
                ts = th * THW
                w0_sb = wrk.tile([128, NCH * 128], BF16, tag="w", bufs=3,
                                 name=f"w0_{th}")
                for k in range(8):
                    nc.sync.dma_start(
                        w0_sb[:, k * 512:(k + 1) * 512],
                        wqkv[CB_ORDER[0], :, k * 512:(k + 1) * 512])
                cos_t = wrk.tile([HD, THW], F32, tag="cos", bufs=1,
                                 name=f"cos_{th}")
                sin_t = wrk.tile([HD, THW], F32, tag="sin", bufs=1,
                                 name=f"sin_{th}")
                nc.sync.dma_start(cos_t[:, :256], cosT[:, ts:ts + 256])
                nc.sync.dma_start(cos_t[:, 256:], cosT[:, ts + 256:ts + THW])
                nc.sync.dma_start(sin_t[:, :256], sinTr[:, ts:ts + 256])
                nc.sync.dma_start(sin_t[:, 256:], sinTr[:, ts + 256:ts + THW])
                # hidden^T block [D, 512] as 8 sub-tiles of 4 c-chunks,
                # with the second proj-weight chunk interleaved mid-stream
                # so cb=1 is not starved behind the hidden-state bulk.
                hts = []
                w1_sb = None
                for j in range(8):
                    t = one.tile([128, 4 * THW], BF16, tag=f"hT{j}",
                                 name=f"hts_{th}_{j}")
                    for k in range(4):
                        nc.sync.dma_start(
                            t[:, k * 512:(k + 1) * 512], hT[th, 4 * j + k])
                    hts.append(t)
                    if j == 3:
                        w1_sb = wrk.tile([128, NCH * 128], BF16, tag="w",
                                         bufs=3, name=f"w1_{th}")
                        for k in range(4):
                            nc.sync.dma_start(
                                w1_sb[:, k * 1024:(k + 1) * 1024],
                                wqkv[CB_ORDER[1], :, k * 1024:(k + 1) * 1024])
                if th == 0:
                    nc.sync.dma_start(ident_sb[:], ident[:])
                    nc.sync.dma_start(ones_sb[:], ones[:])
                    if mask_sb is not None:
                        for k in range(4):
                            nc.sync.dma_start(
                                mask_sb[:, k * 512:(k + 1) * 512],
                                maskT[:, k * 512:(k + 1) * 512])
                qT_lo = wrk.tile([128, 4 * 512], BF16, tag="qTbl",
                                 name=f"qlo_{th}")
                qT_hi = wrk.tile([128, 4 * 512], BF16, tag="qTbh",
                                 name=f"qhi_{th}")
                qT_store[th] = (qT_lo, qT_hi)

                def post(cb, ps):
                    if cb < 10:
                        # RoPE: out = x*cos + swap_halves(x)*sin_signed
                        m1 = wrk.tile([128, THW], F32, tag="m1",
                                      name=f"rm1_{th}_{cb}")
                        nc.vector.tensor_mul(m1[:], ps[:], cos_t[:])
                        m2 = wrk.tile([128, THW], F32, tag="m2",
                                      name=f"rm2_{th}_{cb}")
                        nc.vector.tensor_mul(m2[0:64, :], ps[64:128, :], sin_t[0:64, :])
                        nc.vector.tensor_mul(m2[64:128, :], ps[0:64, :], sin_t[64:128, :])
                        if cb < 8:
                            qdst = qT_lo if cb < 4 else qT_hi
                            nc.vector.tensor_add(
                                qdst[:, (cb % 4) * 512:(cb % 4 + 1) * 512],
                                m1[:], m2[:])
                        else:
                            kv = cb - 8
                            nc.vector.tensor_add(
                                kT_sb[:, kv * TOK + ts: kv * TOK + ts + THW],
                                m1[:], m2[:])
                    else:
                        kv = cb - 10
                        vT = wrk.tile([128, THW], F32R, tag="vT", bufs=1,
                                      name=f"vT_{th}_{cb}")
                        nc.vector.tensor_copy(vT[:], ps[:])
                        for j in range(THW // 128):
                            tb = th * (THW // 128) + j
                            pt = psp.tile([128, 128], F32R, tag="aux",
                                          name=f"pt_{th}_{cb}_{j}")
                            nc.tensor.transpose(
                                pt[:], vT[:, j * 128:(j + 1) * 128], ident_sb[:])
                            nc.vector.tensor_copy(
                                V_sb[:, tb * KCOLS + kv * 128:
                                     tb * KCOLS + (kv + 1) * 128],
                                pt[:])

                yield
                if th == 0:
                    # proj(0) has nothing to hide its hts DMA behind: run the
                    # two prefetched K chunks' accumulations interleaved
                    # ic-wise so the PE tracks the hidden-state arrivals
                    # instead of stalling on the first chunk's tail.
                    psA = psp.tile([128, THW], F32, tag="pa", name="ps_0_8i")
                    psB = psp.tile([128, THW], F32, tag="pa", name="ps_0_9i")
                    for u in range(8):
                        for v in range(4):
                            ic = 4 * u + v
                            mv = hts[ic // 4][:, (ic % 4) * THW:(ic % 4 + 1) * THW]
                            nc.tensor.matmul(
                                psA[:], w0_sb[:, ic * 128:(ic + 1) * 128], mv,
                                start=(ic == 0), stop=(ic == NCH - 1))
                            nc.tensor.matmul(
                                psB[:], w1_sb[:, ic * 128:(ic + 1) * 128], mv,
                                start=(ic == 0), stop=(ic == NCH - 1))
                        yield
                    post(CB_ORDER[0], psA)
                    yield
                    post(CB_ORDER[1], psB)
                    yield
                    order = list(enumerate(CB_ORDER))[2:]
                else:
                    order = list(enumerate(CB_ORDER))
                for ci, cb in order:
                    if th != 0 and ci == 0:
                        w_sb = w0_sb
                    elif th != 0 and ci == 1:
                        w_sb = w1_sb
                    else:
                        w_sb = wrk.tile([128, NCH * 128], BF16, tag="w",
                                        bufs=3, name=f"w_{th}_{cb}")
                        for k in range(4):
                            nc.sync.dma_start(
                                w_sb[:, k * 1024:(k + 1) * 1024],
                                wqkv[cb, :, k * 1024:(k + 1) * 1024])
                    if th == 1 and ci >= 1:
                        # wo preload rides interleaved behind proj(1)'s
                        # weights; o_proj(0) consumes it nb-major at the
                        # end of attention(0).
                        for k in range(6):
                            ch = (ci - 1) * 6 + k
                            if ch < 64:
                                nb, hc = divmod(ch, 8)
                                nc.sync.dma_start(
                                    wo_sb[:, ch * 512:(ch + 1) * 512],
                                    wo[nb, hc])
                    ps = psp.tile([128, THW], F32, tag="pa",
                                  name=f"ps_{th}_{cb}")
                    for u in range(8):
                        for v in range(4):
                            ic = 4 * u + v
                            nc.tensor.matmul(
                                ps[:],
                                w_sb[:, ic * 128:(ic + 1) * 128],
                                hts[ic // 4][:, (ic % 4) * THW:(ic % 4 + 1) * THW],
                                start=(ic == 0),
                                stop=(ic == NCH - 1),
                            )
                        yield
                    post(cb, ps)
                    yield

            def attention_group(hs, qb, qT_aps, fill):
                """Zipped scoresT/softmax/AV^T for q heads hs, query block qb.
                Zipping two heads gives the scalar-engine exp a full
                matmul's worth of lead time before AV consumes it; `fill`
                ([gen, units_per_step_float, acc]) interleaves pure-PE filler
                units so exp latency never idles the PE."""
                qs = qb * 512
                nkb = 4 * qb + 4 if variant == "causal" else TOK // 128
                n = len(hs)
                att_ps = [psp.tile([128, 512], F32, tag="aux", name=f"att_{h}_{qb}")
                          for h in hs]
                sums = [psp.tile([1, 512], F32, tag="sum", name=f"sum_{h}_{qb}")
                        for h in hs]

                def emit_av(i, kb, expT, co):
                    h = hs[i]
                    kv = h // (QH // 2)
                    nc.tensor.matmul(
                        att_ps[i][:, co:],
                        V_sb[:, kb * KCOLS + kv * 128: kb * KCOLS + (kv + 1) * 128],
                        expT[:, co:],
                        start=(kb == 0), stop=(kb == nkb - 1))
                    nc.tensor.matmul(
                        sums[i][:, co:], ones_sb[:], expT[:, co:],
                        start=(kb == 0), stop=(kb == nkb - 1))

                pend = [[] for _ in range(n)]
                for kb in range(nkb):
                    if variant == "causal" and kb > 4 * qb:
                        co = (kb - 4 * qb) * 128
                    else:
                        co = 0
                    exps = []
                    for i, h in enumerate(hs):
                        kv = h // (QH // 2)
                        # qb 3 has no proj filler, so the pa psum banks are
                        # idle; alternating them in doubles the score
                        # pipeline depth where exp-pacing is tightest.
                        stag = "pa" if (qb == 3 and kb % 2 == 1) else "pb"
                        s_ps = psp.tile([128, 512], F32, tag=stag,
                                        name=f"s_{h}_{qb}_{kb}")
                        nc.tensor.matmul(
                            s_ps[:, co:],
                            kT_sb[:, kv * TOK + kb * 128: kv * TOK + (kb + 1) * 128],
                            qT_aps[i][:, co:],
                            start=True, stop=True)
                        exp_in = s_ps
                        if variant == "causal" and kb >= 4 * qb:
                            o = kb - 4 * qb
                            msk = wrk.tile([128, 512], F32, tag="m1",
                                           name=f"msk_{h}_{qb}_{kb}")
                            nc.vector.tensor_add(
                                msk[:, co:], s_ps[:, co:],
                                mask_sb[:, o * 512 + co:(o + 1) * 512])
                            exp_in = msk
                        elif variant == "general":
                            mt = wrk.tile([128, 512], F32, tag="mt",
                                          name=f"mt_{h}_{qb}_{kb}")
                            nc.sync.dma_start(
                                mt[:], maskT[kb * 128:(kb + 1) * 128, qs:qs + 512])
                            msk = wrk.tile([128, 512], F32, tag="m1",
                                           name=f"mskg_{h}_{qb}_{kb}")
                            nc.vector.tensor_add(msk[:], s_ps[:], mt[:])
                            exp_in = msk
                        expT = wrk.tile([128, 512], BF16, tag="expT", bufs=6,
                                        name=f"exp_{h}_{qb}_{kb}")
                        nc.scalar.activation(
                            expT[:, co:], exp_in[:, co:], EXP, scale=float(SCALE))
                        exps.append(expT)
                    for i in range(n):
                        pend[i].append((kb, exps[i], co))
                        if len(pend[i]) > 1:
                            emit_av(i, *pend[i].pop(0))
                    if fill[0] is not None:
                        fill[2] += fill[1]
                        while fill[2] >= 1.0:
                            fill[2] -= 1.0
                            if next(fill[0], _DONE) is _DONE:
                                fill[0] = None
                                break
                for i in range(n):
                    for e in pend[i]:
                        emit_av(i, *e)
                for i, h in enumerate(hs):
                    recip = wrk.tile([1, 512], F32, tag="rcp",
                                     name=f"rcp_{h}_{qb}")
                    nc.vector.reciprocal_approx_fast(recip[:], sums[i][:])
                    rb = wrk.tile([128, 512], F32, tag="m2",
                                  name=f"rb_{h}_{qb}")
                    nc.gpsimd.partition_broadcast(rb[:], recip[:])
                    at2 = wrk.tile([128, 512], BF16, tag=f"at2_{h}",
                                   name=f"at2_{h}_{qb}")
                    nc.vector.tensor_mul(at2[:], att_ps[i][:], rb[:])
                    at2s[h] = at2

            def o_proj_units(qb, nbs, snap):
                """Row-parallel o_proj partial for query block qb: consumes
                snap[h] ([128 hd, 512 tok] bf16, this core's 8 heads) against
                SBUF-resident wo, writes out[qb*512:(qb+1)*512, nbs cols].
                The first two groups soft-start (h0-h6 queued before either
                h7) so the PE has work while head 7's normalize chain
                completes."""
                def emit_part(o_ps, nb, tb, h0, h1):
                    for h in range(h0, h1):
                        nc.tensor.matmul(
                            o_ps[:],
                            snap[h][:, tb * 128:(tb + 1) * 128],
                            wo_sb[:, (nb * 8 + h) * 512:(nb * 8 + h + 1) * 512],
                            start=(h == 0), stop=(h == QH - 1))

                def finish(o_ps, nb, tb):
                    ot = wrk.tile([128, 512], BF16, tag="ot", bufs=6,
                                  name=f"ot_{qb}_{nb}_{tb}")
                    nc.vector.tensor_copy(ot[:], o_ps[:])
                    r0 = qb * 512 + tb * 128
                    nc.scalar.dma_start(
                        out[r0:r0 + 128, nb * 512:nb * 512 + 256],
                        ot[:, :256])
                    nc.scalar.dma_start(
                        out[r0:r0 + 128, nb * 512 + 256:(nb + 1) * 512],
                        ot[:, 256:])

                groups = [(nb, tb) for nb in nbs for tb in range(4)]
                g0, g1 = groups[0], groups[1]
                ps0 = psp.tile([128, 512], F32, tag="pb",
                               name=f"o_{qb}_{g0[0]}_{g0[1]}")
                ps1 = psp.tile([128, 512], F32, tag="pb",
                               name=f"o_{qb}_{g1[0]}_{g1[1]}")
                emit_part(ps0, *g0, 0, QH - 1)
                emit_part(ps1, *g1, 0, QH - 1)
                emit_part(ps0, *g0, QH - 1, QH)
                finish(ps0, *g0)
                yield
                emit_part(ps1, *g1, QH - 1, QH)
                finish(ps1, *g1)
                yield
                for nb, tb in groups[2:]:
                    o_ps = psp.tile([128, 512], F32, tag="pb",
                                    name=f"o_{qb}_{nb}_{tb}")
                    emit_part(o_ps, nb, tb, 0, QH)
                    finish(o_ps, nb, tb)
                    yield

            def drain(gen):
                if gen is not None:
                    for _ in gen:
                        pass

            def attention_block(th, fill):
                qlo, qhi = qT_store[th]
                for hp in range(0, QH, 2):
                    qsrc = qlo if hp < 4 else qhi
                    attention_group(
                        [hp, hp + 1], th,
                        [qsrc[:, (hp % 4) * 512:(hp % 4 + 1) * 512],
                         qsrc[:, (hp % 4 + 1) * 512:(hp % 4 + 2) * 512]],
                        fill)
                drain(fill[0])

            if variant == "causal":
                # proj(0): emit k/v + the first q pair, then let attention(0)
                # pairs interleave the remaining q chunks (and proj(1)) as
                # filler through proj(0)'s DMA-paced region.
                # units: preamble 1 + interleaved K block (8+2) + 4 cbs
                # (v,v,q0,q1) x 9 -> pair (0,1) ready after 47
                g0 = proj_units(0)
                for _ in range(47):
                    next(g0)
                oproj_defer = None
                n_units = 1 + NCB * 9
                for th in range(NTH):
                    steps = 4 * (4 * th + 4)
                    if th == 0:
                        fill = [itertools.chain(g0, proj_units(1)),
                                (6 * 9 + n_units) / (steps + 6), 0.0]
                    elif th < 3:
                        fill = [proj_units(th + 1), n_units / (steps + 6), 0.0]
                    else:
                        fill = [oproj_defer, 32.0 / (steps + 4), 0.0]
                    attention_block(th, fill)
                    if th == 2:
                        oproj_defer = o_proj_units(2, range(8), dict(at2s))
                    else:
                        drain(o_proj_units(th, range(8), dict(at2s)))
            else:
                for th in range(NTH):
                    ts = th * THW
                    drain(proj_units(th))
                    for qi, qt in ((0, qT_store[th][0]), (1, qT_store[th][1])):
                        nc.scalar.dma_start(
                            qT_spill[qi * 512:(qi + 1) * 512, ts:ts + THW]
                            .rearrange("(i p) t -> p i t", p=128),
                            qt[:].rearrange("p (i t) -> p i t", i=4),
                        )
                for qb in range(4):
                    qts = {}
                    for h in range(QH):
                        qT_t = wrk.tile([128, 512], BF16, tag="qTs",
                                        name=f"qt_{h}_{qb}")
                        nc.sync.dma_start(
                            qT_t[:],
                            qT_spill[h * 128:(h + 1) * 128,
                                     qb * 512:(qb + 1) * 512])
                        qts[h] = qT_t
                    nofill = [None, 0.0, 0.0]
                    for hp in range(0, QH, 2):
                        attention_group([hp, hp + 1], qb,
                                        [qts[hp], qts[hp + 1]], nofill)
                    drain(o_proj_units(qb, range(8), dict(at2s)))

    nc.compile()
    return nc


def _get_program(variant: str):
    if variant not in _PROGRAMS:
        _PROGRAMS[variant] = _build_program(variant)
    return _PROGRAMS[variant]


def _detect_variant(mask: np.ndarray) -> str:
    m = mask.reshape(mask.shape[-2], mask.shape[-1])
    if not m.any():
        return "zero"
    causal = np.where(
        np.tril(np.ones((S, S), dtype=bool)), np.float32(0.0), np.float32(NEG))
    if np.array_equal(m, causal):
        return "causal"
    return "general"


def _bf16(a):
    return np.ascontiguousarray(a.astype(ml_dtypes.bfloat16))


def kernel(hidden_states, cos, sin, attention_mask, Wq, Wk, Wv, Wo):
    hidden_states = np.asarray(hidden_states, dtype=np.float32)
    cos = np.asarray(cos, dtype=np.float32)
    sin = np.asarray(sin, dtype=np.float32)
    attention_mask = np.asarray(attention_mask, dtype=np.float32)
    Wq = np.asarray(Wq, dtype=np.float32)
    Wk = np.asarray(Wk, dtype=np.float32)
    Wv = np.asarray(Wv, dtype=np.float32)
    Wo = np.asarray(Wo, dtype=np.float32)

    variant = _detect_variant(attention_mask)
    nc = _get_program(variant)

    ident = np.eye(128, dtype=np.float32)
    ones = np.ones((128, 1), dtype=ml_dtypes.bfloat16)

    if variant == "causal":
        i = np.arange(128)[:, None]
        j = np.arange(512)[None, :]
        strips = [
            np.where(i <= j - o * 128, np.float32(0.0), np.float32(NEG / SCALE))
            for o in range(4)
        ]
        maskT = np.concatenate(strips, axis=1).astype(np.float32)
    elif variant == "general":
        m = attention_mask.reshape(S, S)
        maskT = np.ascontiguousarray(m.T / np.float32(SCALE))
    else:
        maskT = None

    per_batch = {}
    for b in range(B):
        sT = np.ascontiguousarray(sin[b].T)
        sinTr = np.concatenate([-sT[:64], sT[64:]], axis=0)
        hid = hidden_states[b]  # [2048, 4096]
        # [4 th][32 cchunk][128 c][512 tok]
        hT_t = _bf16(
            hid.T.reshape(32, 128, 4, 512).transpose(2, 0, 1, 3))
        per_batch[b] = (hT_t, np.ascontiguousarray(cos[b].T),
                        np.ascontiguousarray(sinTr))

    def _tile_w(W):  # [4096, C] -> [C//128, 128, 32*128]
        C = W.shape[1]
        return W.reshape(32, 128, C // 128, 128).transpose(2, 1, 0, 3) \
            .reshape(C // 128, 128, 32 * 128)

    in_maps = []
    for c in range(NCORES):
        b, g = divmod(c, 4)
        hT_t, cosT, sinTr = per_batch[b]
        wqkv = _bf16(np.concatenate([
            _tile_w(Wq[:, g * QCOLS:(g + 1) * QCOLS]),
            _tile_w(Wk[:, g * KCOLS:(g + 1) * KCOLS]),
            _tile_w(Wv[:, g * KCOLS:(g + 1) * KCOLS]),
        ], axis=0))
        wo_c = Wo[g * QCOLS:(g + 1) * QCOLS, :]  # [1024, 4096]
        wo_t = _bf16(wo_c.reshape(8, 128, 8, 512).transpose(2, 0, 1, 3))
        im = {
            "hT": hT_t,
            "wqkv": wqkv,
            "wo": wo_t,
            "cosT": cosT,
            "sinTr": sinTr,
            "ident": ident,
            "ones": ones,
        }
        if maskT is not None:
            im["maskT"] = maskT
        in_maps.append(im)

    trace = bool(os.environ.get("KERNEL_TRACE"))
    res = run_bass_kernel_spmd(nc, in_maps, core_ids=list(range(NCORES)),
                               trace=trace)
    if trace:
        print(f"HW exec time: {res.exec_time_ns} ns")

    out = np.empty((B, S, D), dtype=np.float32)
    for b in range(B):
        acc = np.zeros((S, D), dtype=np.float64)
        for g in range(4):
            acc += res.results[4 * b + g]["out"].astype(np.float32)
        out[b] = acc.astype(np.float32)
    return out
